# revision 19
# baseline (speedup 1.0000x reference)
"""Cross-modal attention kernel for Trainium2, batch-parallel over 8 NeuronCores.

Problem (per batch element b, one per core):
    xf = x[b] reshaped [C=256, N=4096]
    q = Wq @ xf + bq          [32, N]
    k = Wk @ xf + bk          [32, N]
    v = Wv @ xf + bv          [256, N]
    attn = softmax_n2(q^T k)  [N, N]
    out = gamma * (v @ attn^T) + x[b]

Device algorithm (per core):
  - QK packed: one matmul pass with lhsT=[WqT|WkT] -> qk [64, N].
  - vT computed directly transposed: vT[m, c] = sum_c' x[c', m] * (gamma*Wv)[c, c'],
    with an extra ones-column (col 256) so the AV matmul also produces softmax
    row-sums for free. gamma is folded into Wv on the host; bv is applied after
    normalization (bias of a softmax-weighted average is just + gamma*bv).
  - attnT[m, n] = k^T q computed directly in [m-part, n-free] layout; exp() on
    ScalarE straight out of PSUM. No max-subtraction: |logits| <= ~40 << 88, so
    fp32 exp cannot overflow.
  - AV: outT[n, 0:256] = sum_m E[m, n] * vT[m, c], outT[n, 256] = rowsum[n].
    Normalize by per-partition reciprocal of the rowsum column, transpose back
    to [c, n] on the PE, add residual x + gamma*bv on VectorE, DMA out.

Matmul operands use float32r (fp32 rounded to 11 mantissa bits by the PE, 4x
faster than fp32); accumulation stays fp32 in PSUM. The residual path reads
the raw fp32 bytes (f32r tiles hold unrounded values; rounding happens inside
the PE), so x passes through exactly.
"""

import numpy as np

import concourse.bacc as bacc
import concourse.tile as tile
from concourse import mybir
from concourse.bass_utils import run_bass_kernel_spmd

B, C, C8, N = 8, 256, 32, 4096
NBLK = 512          # n-block (columns of attention processed per E buffer)
NBLOCKS = N // NBLK
MGRP = 2            # m-tiles fused per PSUM group / exp instruction
MT = N // 128       # number of m-tiles (32)
F32 = mybir.dt.float32
F32R = mybir.dt.float32r
BF16 = mybir.dt.bfloat16

_cache = {}


def _build():
    if "nc" in _cache:
        return _cache["nc"]

    nc = bacc.Bacc("TRN2", target_bir_lowering=False, debug=False, num_devices=8)

    x_d = nc.dram_tensor("x", [C, N], F32R, kind="ExternalInput").ap()
    qkw_d = nc.dram_tensor("qkw", [2, 128, 2 * C8], F32R, kind="ExternalInput").ap()
    qkb_d = nc.dram_tensor("qkb", [2, C8, 1], F32, kind="ExternalInput").ap()
    wvt_d = nc.dram_tensor("wvt", [2, 128, C], F32R, kind="ExternalInput").ap()
    gbv_d = nc.dram_tensor("gbv", [2, 128, 1], F32, kind="ExternalInput").ap()
    idn_d = nc.dram_tensor("idn", [128, 128], F32, kind="ExternalInput").ap()
    out_d = nc.dram_tensor("out", [C, N], F32, kind="ExternalOutput").ap()

    with tile.TileContext(nc) as tc:
        with (
            tc.tile_pool(name="const", bufs=1) as cpool,
            tc.tile_pool(name="xbuf", bufs=1) as xpool,
            tc.tile_pool(name="qk", bufs=1) as qkpool,
            tc.tile_pool(name="vt", bufs=MT) as vtpool,
            tc.tile_pool(name="ebuf", bufs=24) as epool,
            tc.tile_pool(name="small", bufs=4) as spool,
            tc.tile_pool(name="fin", bufs=4) as fpool,
        ):
            qkw = cpool.tile([128, 2, 2 * C8], F32R, tag="qkw")
            qkb = cpool.tile([C8, 2], F32, tag="qkb")
            wvt = cpool.tile([128, 2, C], F32R, tag="wvt")
            gbv = cpool.tile([128, 2], F32, tag="gbv")
            idn = cpool.tile([128, 128], F32, tag="idn")
            nc.sync.dma_start(qkw[:], qkw_d.rearrange("a p d -> p a d"))
            nc.sync.dma_start(qkb[:], qkb_d.rearrange("a p one -> p (a one)"))
            nc.sync.dma_start(wvt[:], wvt_d.rearrange("a p d -> p a d"))
            nc.sync.dma_start(gbv[:], gbv_d.rearrange("a p one -> p (a one)"))
            nc.sync.dma_start(idn[:], idn_d[:])

            ones = cpool.tile([128, 2], F32, tag="ones")
            nc.vector.memset(ones[:], 1.0)

            xa = xpool.tile([128, N], F32R, tag="xa")
            xb = xpool.tile([128, N], F32R, tag="xb")
            for i in range(8):
                s = slice(i * 512, (i + 1) * 512)
                nc.sync.dma_start(xa[:, s], x_d[0:128, s])
                nc.sync.dma_start(xb[:, s], x_d[128:256, s])

            q_t = qkpool.tile([C8, N], F32R, tag="q")
            k_t = qkpool.tile([C8, N], F32R, tag="k")

            # ---- stage A: q = Wq @ x + bq, k = Wk @ x + bk   [32, N] each ----
            # ---- stage B: vT[m] = (x[:, m-tile])^T @ (gamma*Wv)^T, ones col ----
            with tc.tile_pool(name="pab", bufs=2, space="PSUM") as pab:
                for i in range(8):
                    s = slice(i * 512, (i + 1) * 512)
                    for dst, col in ((q_t, 0), (k_t, 1)):
                        pq = pab.tile([C8, 512], F32, tag="ab")
                        nc.tensor.matmul(
                            pq[:], qkw[:, 0, col * C8 : (col + 1) * C8], xa[:, s],
                            start=True, stop=False,
                        )
                        nc.tensor.matmul(
                            pq[:], qkw[:, 1, col * C8 : (col + 1) * C8], xb[:, s],
                            start=False, stop=True,
                        )
                        nc.scalar.activation(
                            dst[:, s], pq[:], mybir.ActivationFunctionType.Identity,
                            bias=qkb[:, col : col + 1],
                        )

                vts = []
                for m in range(MT):
                    s = slice(m * 128, (m + 1) * 128)
                    pv = pab.tile([128, C], F32, tag="ab")
                    nc.tensor.matmul(pv[:], xa[:, s], wvt[:, 0, :], start=True, stop=False)
                    nc.tensor.matmul(pv[:], xb[:, s], wvt[:, 1, :], start=False, stop=True)
                    vt = vtpool.tile([128, C + 2], BF16, tag="vt")
                    nc.vector.tensor_copy(vt[:, 0:C], pv[:])
                    nc.vector.tensor_copy(vt[:, C : C + 2], ones[:])
                    vts.append(vt)

            q_ap = q_t[:]
            k_ap = k_t[:]

            with (
                tc.tile_pool(name="pa", bufs=3, space="PSUM") as pa,
                tc.tile_pool(name="pot", bufs=2, space="PSUM") as pot,
            ):
                for nb in range(NBLOCKS):
                    ns = slice(nb * NBLK, (nb + 1) * NBLK)
                    # ---- stage C: E = exp(k^T q) for this n-block ----
                    eblk = []
                    for mg in range(MT // MGRP):
                        pat = pa.tile([128, MGRP, NBLK], F32, tag="pa")
                        for j in range(MGRP):
                            m = mg * MGRP + j
                            nc.tensor.matmul(
                                pat[:, j, :],
                                k_ap[:, m * 128 : (m + 1) * 128],
                                q_ap[:, ns],
                                start=True,
                                stop=True,
                            )
                        et = epool.tile([128, MGRP, NBLK], BF16, tag="e")
                        nc.scalar.activation(et[:], pat[:], mybir.ActivationFunctionType.Exp)
                        eblk.append(et)

                    # ---- stage D: outT = E^T-weighted sums of vT (+rowsum col) ----
                    for t in range(NBLK // 128):
                        po = pot.tile([128, C + 2], F32, tag="ot")
                        for mg in range(MT // MGRP):
                            for j in range(MGRP):
                                m = mg * MGRP + j
                                nc.tensor.matmul(
                                    po[:],
                                    eblk[mg][:, j, t * 128 : (t + 1) * 128],
                                    vts[m][:],
                                    start=(m == 0),
                                    stop=(m == MT - 1),
                                )
                        inv = spool.tile([128, 1], F32, tag="inv")
                        nc.vector.reciprocal(inv[:], po[:, C : C + 1])
                        ot = spool.tile([128, C], F32, tag="ot_sb")
                        nc.vector.tensor_scalar(ot[:], po[:, 0:C], inv[:], None, mybir.AluOpType.mult)

                        # ---- stage E: transpose to [c, n], add residual, store ----
                        gn = nb * NBLK + t * 128
                        for cc, xres in ((0, xa), (1, xb)):
                            pt = pot.tile([128, 128], F32, tag="ot")
                            nc.tensor.transpose(pt[:], ot[:, cc * 128 : (cc + 1) * 128], idn[:])
                            fin = fpool.tile([128, 128], F32, tag="fin")
                            nc.vector.tensor_tensor(
                                fin[:],
                                pt[:],
                                xres[:, gn : gn + 128].bitcast(F32),
                                mybir.AluOpType.add,
                            )
                            nc.vector.tensor_scalar(
                                fin[:], fin[:], gbv[:, cc : cc + 1], None, mybir.AluOpType.add
                            )
                            nc.sync.dma_start(
                                out_d[cc * 128 : (cc + 1) * 128, gn : gn + 128], fin[:]
                            )

    nc.compile()
    _cache["nc"] = nc
    return nc


last_results = None


def kernel(x, Wq, bq, Wk, bk, Wv, bv, gamma, trace=False):
    global last_results
    nc = _build()

    x = np.ascontiguousarray(np.asarray(x, dtype=np.float32)).reshape(B, C, N)
    Wq = np.asarray(Wq, dtype=np.float32)
    Wk = np.asarray(Wk, dtype=np.float32)
    Wv = np.asarray(Wv, dtype=np.float32)
    g = np.float32(np.asarray(gamma, dtype=np.float32).reshape(-1)[0])

    qkw = np.ascontiguousarray(
        np.concatenate([Wq.T, Wk.T], axis=1).reshape(2, 128, 2 * C8)
    )
    qkb = np.stack(
        [np.asarray(bq, np.float32), np.asarray(bk, np.float32)]
    ).reshape(2, C8, 1)
    wvt = np.ascontiguousarray((g * Wv).T.reshape(2, 128, C))
    gbv = (g * np.asarray(bv, np.float32)).reshape(2, 128, 1)
    idn = np.eye(128, dtype=np.float32)

    shared = {"qkw": qkw, "qkb": qkb, "wvt": wvt, "gbv": gbv, "idn": idn}
    in_maps = [{"x": x[b], **shared} for b in range(B)]

    res = run_bass_kernel_spmd(nc, in_maps, core_ids=list(range(B)), trace=trace)
    last_results = res
    out = np.stack([res.results[b]["out"] for b in range(B)], axis=0)
    return out.reshape(B, C, 64, 64)
